# revision 1
# baseline (speedup 1.0000x reference)
"""Chamfer loss kernel for Trainium2 (8 NeuronCores, data-parallel over batch).

Math:
  For each batch b: P[i,j] = |x_i|^2 + |y_j|^2 - 2 x_i.y_j  (x=preds[b].T, y=gts[b].T)
  loss_b = sum_j min_i P + sum_i min_j P ; output = sum_b loss_b.

  On device we compute PN = -P/2 via a single K=9 matmul:
    lhsT rows: [x0, x1, x2, x0^2, x1^2, x2^2, -1/2, -1/2, -1/2]
    rhs  rows: [y0, y1, y2, -1/2, -1/2, -1/2, y0^2, y1^2, y2^2]
    PN[i,j] = x.y - |x|^2/2 - |y|^2/2
  min_i P = -2 max_i PN, so loss_b = -2 * (sum_j max_i PN + sum_i max_j PN).

  max commutes with blocking: row/col maxes are accumulated as *elementwise*
  tensor_tensor max ops over tiles (fp16, DVE 2x mode); the only true
  reductions are small tails (one reduce_max per 128-row block; partition-max
  of the column accumulators via PE transpose + free-axis reduce).
"""

import os
from contextlib import ExitStack

import numpy as np

import concourse.bacc as bacc
import concourse.bass as bass
import concourse.mybir as mybir
import concourse.tile as tile
from concourse.bass_utils import run_bass_kernel_spmd

B, D, N = 8, 3, 8192
N_CORES = 8

IB = 128          # i-block (output partition dim)
JBW = 1024        # j pair-block width (2 PSUM banks)
N_IB = N // IB    # 64
N_JBP = N // JBW  # 8
POOL_C_JBPS = ()  # set below: which jbp column-accumulators gpsimd handles

F32 = mybir.dt.float32
F16 = mybir.dt.float16
BF16 = mybir.dt.bfloat16
AX = mybir.AxisListType
ALU = mybir.AluOpType

# jbp groups whose C-merge runs on gpsimd (Pool) instead of DVE
POOL_C_JBPS = (4, 5, 6, 7)

_last_results = None  # stash for test harness (exec_time etc.)


def build_kernel(n: int = N):
    """Builds the SPMD Bass program for one core handling one batch."""
    n_ib = n // IB
    n_jbp = n // JBW

    nc = bacc.Bacc("TRN2", target_bir_lowering=False, debug=False)

    preds_d = nc.dram_tensor("preds", [D, n], F32, kind="ExternalInput").ap()
    gts_d = nc.dram_tensor("gts", [D, n], F32, kind="ExternalInput").ap()
    ident_d = nc.dram_tensor("ident", [128, 128], F16, kind="ExternalInput").ap()
    out_d = nc.dram_tensor("out", [1, 1], F32, kind="ExternalOutput").ap()

    with tile.TileContext(nc) as tc, ExitStack() as ctx:
        persist = ctx.enter_context(tc.tile_pool(name="persist", bufs=1))
        spool = ctx.enter_context(tc.tile_pool(name="spool", bufs=12))
        rpool = ctx.enter_context(tc.tile_pool(name="rpool", bufs=3))

        # ---- prologue: build XT [21, n] and YT [21, n] (bf16 hi/lo split) ----
        # The PE's fp32 path is ~8x slower and its fp16 path is inexact, so
        # inputs are bf16 hi/lo pairs giving exact products:
        #   x.y ~ hx.hy + hx.ly + lx.hy   (lo.lo term ~2^-18, dropped)
        #   x^2 as hsq + lsq (bf16 pair of the fp32 square)
        # Row pairing (lhsT row k multiplies rhs row k):
        #   k 0-2 : hx_d   | hy_d        k 9-11 : hsqx_d | -1/2
        #   k 3-5 : hx_d   | ly_d        k 12-14: lsqx_d | -1/2
        #   k 6-8 : lx_d   | hy_d        k 15-17: -1/2   | hsqy_d
        #                                k 18-20: -1/2   | lsqy_d
        # Prologue math runs in a [96, n/32] layout (partition p = d*32 + c,
        # chunk c of 32) so all DVE lanes are used; DMAs scatter rows into
        # place afterwards.
        XT = persist.tile([21, n], BF16)
        YT = persist.tile([21, n], BF16)
        ident = persist.tile([128, 128], F16)
        nc.sync.dma_start(ident[:], ident_d[:])
        fw = n // 32
        with tc.tile_pool(name="propool", bufs=1) as propool:
            # const -1/2 everywhere first (aligned base-0 memset); data rows
            # are DMA-scattered over it, leaving the const rows at -1/2.
            nc.gpsimd.memset(XT[:], -0.5)
            nc.gpsimd.memset(YT[:], -0.5)
            for src_d, T, rows in ((preds_d, XT, (0, 3, 6, 9, 12)),
                                   (gts_d, YT, (0, 6, 3, 15, 18))):
                # rows = (hi, hi_dup, lo, hsq, lsq) destination row starts
                nm = "x" if T is XT else "y"
                P = propool.tile([96, fw], F32, name=f"P{nm}")
                H = propool.tile([96, fw], BF16, name=f"H{nm}")
                L = propool.tile([96, fw], BF16, name=f"L{nm}")
                SQ = propool.tile([96, fw], F32, name=f"SQ{nm}")
                HS = propool.tile([96, fw], BF16, name=f"HS{nm}")
                LS = propool.tile([96, fw], BF16, name=f"LS{nm}")
                nc.sync.dma_start(P[:], src_d.rearrange("d (c f) -> (d c) f", c=32))
                nc.scalar.copy(H[:], P[:])
                nc.vector.tensor_tensor(out=L[:], in0=P[:], in1=H[:], op=ALU.subtract)
                nc.vector.tensor_tensor(out=SQ[:], in0=P[:], in1=P[:], op=ALU.mult)
                nc.scalar.copy(HS[:], SQ[:])
                nc.vector.tensor_tensor(out=LS[:], in0=SQ[:], in1=HS[:], op=ALU.subtract)
                for t, r in zip((H, H, L, HS, LS), rows):
                    for d in range(D):
                        nc.sync.dma_start(
                            T[r + d:r + d + 1, :].rearrange("p (c f) -> p c f", c=32),
                            t[d * 32:(d + 1) * 32, :],
                        )

        # ---- main loop ----
        # PSUM is consumed in 4-bank quads: 4 matmuls fill [128, 2048], one
        # ACT op exits the whole quad to SBUF fp16. DVE then does one
        # quad-wide column merge and two half-wide row merges per quad.
        QW = min(2048, n)
        n_q = n // QW
        rw = min(JBW, QW)  # row-accumulator width

        # persistent accumulators
        C = [persist.tile([128, QW], F16, name=f"C{j}") for j in range(n_q)]
        rowmaxes = persist.tile([128, n_ib], F32)

        psum_ctx = tc.tile_pool(name="psum", bufs=2, space=bass.MemorySpace.PSUM)
        psum = psum_ctx.__enter__()
        MMW = min(512, QW)  # fp32 PSUM output limits one matmul to one bank
        for ib in range(n_ib):
            lhsT = XT[:, ib * IB:(ib + 1) * IB]
            squads = []
            for q in range(n_q):
                p = psum.tile([128, QW], F32, tag="p")
                for m in range(QW // MMW):
                    c0 = q * QW + m * MMW
                    nc.tensor.matmul(
                        p[:, m * MMW:(m + 1) * MMW], lhsT, YT[:, c0:c0 + MMW],
                        start=True, stop=True,
                    )
                s = spool.tile([128, QW], F16, tag="s")
                nc.scalar.copy(s[:], p[:])  # PSUM exit + f32->f16
                squads.append(s)
                # column accumulator: one quad-wide merge
                if ib == 0:
                    nc.vector.tensor_copy(C[q][:], s[:])
                else:
                    nc.vector.tensor_tensor(out=C[q][:], in0=C[q][:], in1=s[:], op=ALU.max)
            # row accumulator: pairwise tree over the quads, then fold + reduce
            R = rpool.tile([128, QW], F16, tag="R")
            if n_q >= 4:
                R2 = rpool.tile([128, QW], F16, tag="R2")
                nc.vector.tensor_tensor(out=R[:], in0=squads[0][:], in1=squads[1][:], op=ALU.max)
                nc.vector.tensor_tensor(out=R2[:], in0=squads[2][:], in1=squads[3][:], op=ALU.max)
                nc.vector.tensor_tensor(out=R[:], in0=R[:], in1=R2[:], op=ALU.max)
            elif n_q == 2:
                nc.vector.tensor_tensor(out=R[:], in0=squads[0][:], in1=squads[1][:], op=ALU.max)
            else:
                nc.vector.tensor_copy(R[:], squads[0][:])
            if QW > rw:
                nc.vector.tensor_tensor(
                    out=R[:, 0:rw], in0=R[:, 0:rw], in1=R[:, rw:2 * rw], op=ALU.max)
            nc.vector.tensor_reduce(
                out=rowmaxes[:, ib:ib + 1], in_=R[:, 0:rw], axis=AX.X, op=ALU.max
            )

        psum_ctx.__exit__(None, None, None)

        # ---- tails ----
        tailp = ctx.enter_context(
            tc.tile_pool(name="tailp", bufs=2, space=bass.MemorySpace.PSUM)
        )
        # loss2 partial: sum_i max_j  -> [128,1]
        acc2 = persist.tile([128, 1], F32)
        nc.vector.reduce_sum(out=acc2[:], in_=rowmaxes[:], axis=AX.X)

        # loss1: partition-max of every C column via PE transpose (4 chunks
        # batched per PSUM tile, one [128, 4, 128] reduce each), then sum_j
        n_chunks = QW // 128
        n_cols = n // 128
        colmax_cols = persist.tile([128, n_cols], F32)
        for q in range(n_q):
            for g in range(n_chunks // 4):
                pt = tailp.tile([128, 512], F16, tag="pt")
                for c in range(4):
                    ch = g * 4 + c
                    nc.tensor.transpose(
                        pt[:, c * 128:(c + 1) * 128],
                        C[q][:, ch * 128:(ch + 1) * 128], ident[:],
                    )
                idx = q * n_chunks + g * 4
                nc.vector.tensor_reduce(
                    out=colmax_cols[:, idx:idx + 4],
                    in_=pt[:].rearrange("p (c f) -> p c f", c=4),
                    axis=AX.X, op=ALU.max,
                )
        acc1 = persist.tile([128, 1], F32)
        nc.vector.reduce_sum(out=acc1[:], in_=colmax_cols[:], axis=AX.X)

        total = persist.tile([128, 1], F32)
        nc.vector.tensor_tensor(out=total[:], in0=acc1[:], in1=acc2[:], op=ALU.add)

        # partition-sum via matmul with ones, then scale by -2
        ones = persist.tile([128, 1], F32)
        nc.vector.memset(ones[:], 1.0)
        ps = tailp.tile([1, 1], F32, tag="ps")
        nc.tensor.matmul(ps[:], ones[:], total[:], start=True, stop=True)
        out_sb = persist.tile([1, 1], F32)
        nc.scalar.mul(out_sb[:], ps[:], -2.0)
        nc.sync.dma_start(out_d[:], out_sb[:])

    nc.compile()
    return nc


def kernel(preds: np.ndarray, gts: np.ndarray) -> np.ndarray:
    global _last_results
    assert preds.shape == (B, D, N) and gts.shape == (B, D, N)
    nc = build_kernel(N)
    eye = np.eye(128, dtype=np.float16)
    in_maps = [
        {
            "preds": np.ascontiguousarray(preds[b], dtype=np.float32),
            "gts": np.ascontiguousarray(gts[b], dtype=np.float32),
            "ident": eye,
        }
        for b in range(N_CORES)
    ]
    res = run_bass_kernel_spmd(
        nc,
        in_maps,
        core_ids=list(range(N_CORES)),
        trace=bool(os.environ.get("BASS_TRACE")),
    )
    _last_results = res
    total = sum(float(res.results[i]["out"].reshape(-1)[0]) for i in range(N_CORES))
    return np.array(total, dtype=np.float32)

